# revision 1
# baseline (speedup 1.0000x reference)
"""ComplexLSTM Trainium2 kernel.

Problem: x [2, 64, 128, 1024] (real/imag, B, I, T) -> out [2, 64, 256, 1024].
Four real LSTM applications: lstm_r(x_real), lstm_r(x_imag), lstm_i(x_real),
lstm_i(x_imag); combined as L_r = r(xr) - i(xim), L_i = r(xim) + i(xr).

Sharding: 2 weight-sets x 128 sequences each = 256 independent sequences.
8 cores x 32 sequences (cores 0-3: r-weights, cores 4-7: i-weights).

Device layout (fully transposed state, weights-stationary matmuls):
  gates.T accumulated in PSUM as [128p, 8 blocks, 32 batch] where
  block j = gate rows 128j..128j+127, gate order permuted to [g,g,i,i,f,f,o,o].
  bias enters via an indicator matmul (lhsT=bias [8,128], rhs=onehot [8,256]),
  x-projection via 8 MMs (lhsT=WihT tiles, rhs=x_t.T), recurrent part via
  16 MMs (lhsT=WhhT tiles bf16 FWL, rhs=h.T slices of the history buffer).
  Elementwise ops are all [128, 64]-shaped (tiny free dims).
  h.T written bf16 directly into a T-chunk history buffer that doubles as the
  matmul rhs for the next step; chunks DMA'd to HBM; host does the final
  (trivial) combine/transpose.
"""

import numpy as np
import ml_dtypes
from contextlib import ExitStack

import concourse.bass as bass
import concourse.bacc as bacc
import concourse.tile as tile
from concourse import mybir
from concourse.bass_utils import run_bass_kernel_spmd

BF16 = mybir.dt.bfloat16
F32 = mybir.dt.float32
AF = mybir.ActivationFunctionType
OP = mybir.AluOpType

B, I, T_FULL, H = 64, 128, 1024, 256
NB = 32          # batch (sequences) per core
NCORES = 8
TC = 128         # history chunk (steps per output DMA)
XC = 64          # x input chunk (steps per input DMA)

_cache = {}


def build(T):
    nc = bacc.Bacc("TRN2", target_bir_lowering=False, debug=False)

    tc_hist = max(1, min(TC, T))
    xc = max(1, min(XC, T))
    assert T % tc_hist == 0 and T % xc == 0

    xT_d = nc.declare_dram_parameter("xT", [128, T, NB], BF16, isOutput=False)
    whhT_d = nc.declare_dram_parameter("whhT", [128, 2, 8, 128], BF16, isOutput=False)
    wihT_d = nc.declare_dram_parameter("wihT", [128, 8, 128], BF16, isOutput=False)
    biasK_d = nc.declare_dram_parameter("biasK", [8, 128], BF16, isOutput=False)
    ind_d = nc.declare_dram_parameter("ind", [8, 8 * NB], BF16, isOutput=False)
    hist_d = nc.declare_dram_parameter("hist", [128, 2, NB, T], BF16, isOutput=True)

    with tile.TileContext(nc) as tc, ExitStack() as ctx:
        consts = ctx.enter_context(tc.tile_pool(name="consts", bufs=1))
        xin = ctx.enter_context(tc.tile_pool(name="xin", bufs=2))
        hpool = ctx.enter_context(tc.tile_pool(name="hist", bufs=2))
        psum = ctx.enter_context(tc.tile_pool(name="psum", bufs=2, space="PSUM"))
        sml = ctx.enter_context(tc.tile_pool(name="small", bufs=3))
        cpool = ctx.enter_context(tc.tile_pool(name="cpool", bufs=3))

        WHH = consts.tile([128, 2, 8, 128], BF16)
        nc.sync.dma_start(WHH[:], whhT_d[:])
        WIH = consts.tile([128, 8, 128], BF16)
        nc.sync.dma_start(WIH[:], wihT_d[:])
        BIASK = consts.tile([8, 128], BF16)
        nc.sync.dma_start(BIASK[:], biasK_d[:])
        IND = consts.tile([8, 8 * NB], BF16)
        nc.sync.dma_start(IND[:], ind_d[:])

        XBUF = None
        HIST = None
        c_prev = None
        h_prev = None  # AP into HIST for h.T(t-1)

        for t in range(T):
            tl = t % xc
            if tl == 0:
                XBUF = xin.tile([128, xc, NB], BF16, tag="xbuf")
                nc.sync.dma_start(XBUF[:], xT_d[:, t:t + xc, :])
            th = t % tc_hist
            if th == 0:
                HIST = hpool.tile([128, 2, NB, tc_hist], BF16, tag="hist")

            g_ps = psum.tile([128, 8, NB], F32, tag="gates")
            # bias (clears PSUM), then x-projection, then recurrent part
            nc.tensor.matmul(g_ps[:], BIASK[:], IND[:], start=True, stop=False)
            for m in range(8):
                nc.tensor.matmul(
                    g_ps[:, m, :], WIH[:, m, :], XBUF[:, tl, :],
                    start=False, stop=(t == 0 and True) and False,
                )
            if t > 0:
                for m in range(8):
                    for k in range(2):
                        nc.tensor.matmul(
                            g_ps[:, m, :], WHH[:, k, m, :], h_prev[:, k, :],
                            start=False, stop=(k == 1),
                        )

            # activations: blocks [0:2]=g (tanh), [2:8]=i,f,o (sigmoid)
            sg = sml.tile([128, 6, NB], F32, tag="sg")
            nc.scalar.activation(sg[:], g_ps[:, 2:8, :], AF.Sigmoid)
            gt = sml.tile([128, 2, NB], F32, tag="gt")
            nc.scalar.activation(gt[:], g_ps[:, 0:2, :], AF.Tanh)

            v = sml.tile([128, 2, NB], F32, tag="v")
            nc.vector.tensor_tensor(v[:], sg[:, 0:2, :], gt[:], OP.mult)
            c_new = cpool.tile([128, 2, NB], F32, tag="c")
            if t > 0:
                u = sml.tile([128, 2, NB], F32, tag="u")
                nc.vector.tensor_tensor(u[:], sg[:, 2:4, :], c_prev[:], OP.mult)
                nc.vector.tensor_tensor(c_new[:], u[:], v[:], OP.add)
            else:
                nc.vector.tensor_copy(c_new[:], v[:])
            tch = sml.tile([128, 2, NB], F32, tag="tch")
            nc.scalar.activation(tch[:], c_new[:], AF.Tanh)
            h_slot = HIST[:, :, :, th]
            nc.vector.tensor_tensor(h_slot, sg[:, 4:6, :], tch[:], OP.mult)

            c_prev = c_new
            h_prev = HIST[:, :, :, th]

            if th == tc_hist - 1:
                t0 = t - (tc_hist - 1)
                nc.sync.dma_start(hist_d[:, :, :, t0:t0 + tc_hist], HIST[:])
    nc.compile()
    return nc


def _get_nc(T):
    if T not in _cache:
        _cache[T] = build(T)
    return _cache[T]


def _prep_core_inputs(x, Wih, Whh, bih, bhh, T):
    """Per weight-set host prep. Returns (shared weight arrays, xT per 4 cores)."""
    # gate permutation torch [i,f,g,o] -> [g,i,f,o]
    perm = np.concatenate([np.arange(512, 768), np.arange(0, 256),
                           np.arange(256, 512), np.arange(768, 1024)])
    Wihp = np.asarray(Wih)[perm]          # [1024, 128]
    Whhp = np.asarray(Whh)[perm]          # [1024, 256]
    biasp = (np.asarray(bih) + np.asarray(bhh))[perm]  # [1024]

    whhT = Whhp.reshape(8, 128, 2, 128).transpose(3, 2, 0, 1)  # [p,k,m,j]
    wihT = Wihp.reshape(8, 128, 128).transpose(2, 0, 1)        # [p,m,j]
    biasK = biasp.reshape(8, 128)
    whhT = whhT.astype(ml_dtypes.bfloat16)
    wihT = wihT.astype(ml_dtypes.bfloat16)
    biasK = biasK.astype(ml_dtypes.bfloat16)

    # batch-128 for this weight set: seqs 0-63 = x_real (x[0]), 64-127 = x_imag
    # x: [2, B, I, T]; per seq [I, T] slice. xT per core: [128, T, 32]
    xTs = []
    xall = np.concatenate([np.asarray(x)[0], np.asarray(x)[1]], axis=0)  # [128, I, T]
    for g in range(4):
        sl = xall[32 * g:32 * g + 32]             # [32, I, T]
        xT = sl.transpose(1, 2, 0)[:, :T, :]      # [I, T, 32]
        xTs.append(np.ascontiguousarray(xT).astype(ml_dtypes.bfloat16))
    return whhT, wihT, biasK, xTs


def _run(x, Wih_r, Whh_r, bih_r, bhh_r, Wih_i, Whh_i, bih_i, bhh_i, T,
         trace=False, tmpdir=None):
    nc = _get_nc(T)
    ind = np.kron(np.eye(8), np.ones((1, NB))).astype(ml_dtypes.bfloat16)

    whhT_r, wihT_r, biasK_r, xTs_r = _prep_core_inputs(x, Wih_r, Whh_r, bih_r, bhh_r, T)
    whhT_i, wihT_i, biasK_i, xTs_i = _prep_core_inputs(x, Wih_i, Whh_i, bih_i, bhh_i, T)

    in_maps = []
    for core in range(NCORES):
        ws = core // 4
        g = core % 4
        whhT, wihT, biasK = (whhT_r, wihT_r, biasK_r) if ws == 0 else (whhT_i, wihT_i, biasK_i)
        xT = (xTs_r if ws == 0 else xTs_i)[g]
        in_maps.append({
            "xT": xT, "whhT": whhT, "wihT": wihT, "biasK": biasK, "ind": ind,
        })
    res = run_bass_kernel_spmd(nc, in_maps, core_ids=list(range(NCORES)),
                               trace=trace, tmpdir=tmpdir)
    results = res.results

    # reassemble: hist [128, 2, 32, T] -> [H=256, 32, T] per core
    def hmat(ws):
        parts = []
        for g in range(4):
            h = results[4 * ws + g]["hist"].astype(np.float32)
            parts.append(h.transpose(1, 0, 2, 3).reshape(256, NB, T))
        return np.concatenate(parts, axis=1)  # [256, 128, T]

    Hr = hmat(0)
    Hi = hmat(1)
    L_r = Hr[:, 0:64] - Hi[:, 64:128]   # [256, 64, T]
    L_i = Hr[:, 64:128] + Hi[:, 0:64]
    out = np.stack([L_r.transpose(1, 0, 2), L_i.transpose(1, 0, 2)], axis=0)
    return np.ascontiguousarray(out.astype(np.float32)), res


def kernel(x, Wih_r, Whh_r, bih_r, bhh_r, Wih_i, Whh_i, bih_i, bhh_i):
    out, _ = _run(x, Wih_r, Whh_r, bih_r, bhh_r,
                  Wih_i, Whh_i, bih_i, bhh_i, T_FULL)
    return out

